# revision 1
# baseline (speedup 1.0000x reference)
"""MX-quantized Llama attention (B=2,S=1024,H=4096,NH=32,KVH=8,D=128) on 8 trn2 cores.

Sharding: tensor-parallel over heads. Core c owns q-heads [4c,4c+4), kv-head c,
Wo out-feature rows [512c,512c+512). Hidden-state MX quantization is sharded by
tokens (256/core) and AllGathered in fp8; attention outputs are quantized,
transposed and AllGathered in fp8 for the (column-parallel) output projection.
Host does layout-only work: slicing, transposes of cos/sin, final concat.

Quantization pipeline (exact vs the MX reference, validated bit-exact on HW):
  amax->block scales as pow2 bit tricks, y=x*ilo preclamped, round via per-element
  magic constant Cmag=1.5*2^(23-m)*clip(2^e(y),1,4), q=(y+Cmag)-Cmag, out=q*lo.
  Weights carry an extra 2^4 shift (fp8-subnormal safety), compensated at PSUM
  evacuation (scale 2^-4).
"""
import numpy as np

import concourse.bass as bass
import concourse.mybir as mybir
import concourse.tile as tile
from concourse import bacc
from concourse.bass_utils import run_bass_kernel_spmd

F32 = mybir.dt.float32
BF16 = mybir.dt.bfloat16
I32 = mybir.dt.int32
FP8 = mybir.dt.float8e4

MASK_EXP = 0x7F800000
CMAX_ACT = 7.74999952316284180   # largest fp32 < 7.75  (e2m3, maxval 7.5)
CMAX_W = 6.99999952316284180     # largest fp32 < 7.0   (e2m1, maxval 6.0)
WSHIFT = 4
SCALING = float(np.float32(1.0) / np.sqrt(np.float32(128.0)))
NEG_BIG = -1.0e9

B, S, H, NH, KVH, D = 2, 1024, 4096, 32, 8, 128
T = B * S              # 2048 global tokens
TPC = T // 8           # 256 tokens per core
QH = NH // KVH         # 4 q heads per core


def _emit_quant(nc, scratch, x_ap, q8_out, mbits, cmax, shift, pfx):
    """Quantize x_ap [128, F] fp32 (F multiple of 32) -> q8_out [128, F] fp8.

    Output value = q_ref * 2**shift.
    """
    fN = x_ap.shape[-1]
    nblk = fN // 32
    x3 = x_ap.rearrange("p (b j) -> p b j", j=32)

    amax = scratch.tile([128, nblk], F32, tag=pfx + "amax")
    nc.vector.tensor_reduce(out=amax, in_=x3, axis=mybir.AxisListType.X,
                            op=mybir.AluOpType.max, apply_absolute_value=True)
    px_a = scratch.tile([128, nblk], I32, tag=pfx + "px_a")
    nc.vector.tensor_scalar(out=px_a, in0=amax.bitcast(I32),
                            scalar1=MASK_EXP, scalar2=None,
                            op0=mybir.AluOpType.bitwise_and)
    t1 = scratch.tile([128, nblk], I32, tag=pfx + "t1")
    nc.vector.tensor_scalar(out=t1, in0=px_a, scalar1=MASK_EXP, scalar2=None,
                            op0=mybir.AluOpType.bitwise_xor)
    ilo = scratch.tile([128, nblk], F32, tag=pfx + "ilo")
    nc.vector.tensor_scalar(out=ilo.bitcast(I32), in0=t1,
                            scalar1=0x00800000, scalar2=None,
                            op0=mybir.AluOpType.add)
    lo = scratch.tile([128, nblk], F32, tag=pfx + "lo")
    nc.vector.tensor_scalar(out=lo.bitcast(I32), in0=px_a,
                            scalar1=(shift - 2) << 23, scalar2=0,
                            op0=mybir.AluOpType.add, op1=mybir.AluOpType.max)

    y = scratch.tile([128, fN], F32, tag=pfx + "y")
    nc.vector.tensor_tensor(
        out=y.rearrange("p (b j) -> p b j", j=32), in0=x3,
        in1=ilo.unsqueeze(2).broadcast_to([128, nblk, 32]),
        op=mybir.AluOpType.mult)
    yc = scratch.tile([128, fN], F32, tag=pfx + "yc")
    nc.vector.tensor_scalar(out=yc, in0=y, scalar1=cmax, scalar2=-cmax,
                            op0=mybir.AluOpType.min, op1=mybir.AluOpType.max)
    E1 = scratch.tile([128, fN], F32, tag=pfx + "E1")
    nc.vector.tensor_scalar(out=E1.bitcast(I32), in0=yc.bitcast(I32),
                            scalar1=MASK_EXP, scalar2=None,
                            op0=mybir.AluOpType.bitwise_and)
    KK = ((23 - mbits) << 23) + 0x00400000
    A = int(np.float32(1.5 * 2.0 ** (23 - mbits)).view(np.int32))
    Bc = int(np.float32(1.5 * 2.0 ** (25 - mbits)).view(np.int32))
    cm1 = scratch.tile([128, fN], F32, tag=pfx + "cm1")
    nc.vector.tensor_scalar(out=cm1.bitcast(I32), in0=E1.bitcast(I32),
                            scalar1=KK, scalar2=None, op0=mybir.AluOpType.add)
    cmag = scratch.tile([128, fN], F32, tag=pfx + "cmag")
    nc.vector.tensor_scalar(out=cmag.bitcast(I32), in0=cm1.bitcast(I32),
                            scalar1=A, scalar2=Bc,
                            op0=mybir.AluOpType.max, op1=mybir.AluOpType.min)
    s = scratch.tile([128, fN], F32, tag=pfx + "s")
    nc.vector.tensor_tensor(out=s, in0=yc, in1=cmag, op=mybir.AluOpType.add)
    qy = scratch.tile([128, fN], F32, tag=pfx + "qy")
    nc.vector.tensor_tensor(out=qy, in0=s, in1=cmag, op=mybir.AluOpType.subtract)
    nc.vector.tensor_tensor(
        out=q8_out.rearrange("p (b j) -> p b j", j=32),
        in0=qy.rearrange("p (b j) -> p b j", j=32),
        in1=lo.unsqueeze(2).broadcast_to([128, nblk, 32]),
        op=mybir.AluOpType.mult)


def build_kernel(reps=1):
    nc = bacc.Bacc(num_devices=8)

    hid_in = nc.declare_dram_parameter("hid", [TPC, H], F32, isOutput=False)
    wq_in = nc.declare_dram_parameter("wq", [QH * D, H], F32, isOutput=False)
    wk_in = nc.declare_dram_parameter("wk", [D, H], F32, isOutput=False)
    wv_in = nc.declare_dram_parameter("wv", [D, H], F32, isOutput=False)
    wo_in = nc.declare_dram_parameter("wo", [QH * D, H], F32, isOutput=False)
    cos_in = nc.declare_dram_parameter("cosR", [T, D], F32, isOutput=False)
    sin_in = nc.declare_dram_parameter("sinR", [T, D], F32, isOutput=False)
    oT_out = nc.declare_dram_parameter("oT", [QH * D, T], F32, isOutput=True)

    hidT_loc = nc.dram_tensor("hidT_loc", [H, TPC], FP8)
    hidT_full = nc.dram_tensor("hidT_full", [8 * H, TPC], FP8, addr_space="Shared")
    attnT_loc = [nc.dram_tensor(f"attnT_loc{b}", [QH * D, S], FP8) for b in range(B)]
    attnT_full = [nc.dram_tensor(f"attnT_full{b}", [8 * QH * D, S], FP8,
                                 addr_space="Shared") for b in range(B)]

    NKC = H // 128   # 32 contraction chunks

    with tile.TileContext(nc) as tc:
        with (
            tc.tile_pool(name="const", bufs=1) as constp,
            tc.tile_pool(name="scratch", bufs=2) as scratch,
            tc.tile_pool(name="wres", bufs=1) as wres,
            tc.tile_pool(name="qkres", bufs=1) as qkres,
            tc.tile_pool(name="stream", bufs=3) as stream,
            tc.tile_pool(name="evac", bufs=3) as evac,
        ):
            # ---- constants ----
            ident8 = constp.tile([128, 128], FP8)
            nc.gpsimd.memset(ident8, 0.0)
            nc.gpsimd.affine_select(out=ident8, in_=ident8,
                                    compare_op=mybir.AluOpType.not_equal,
                                    fill=1.0, base=0, pattern=[[-1, 128]],
                                    channel_multiplier=1)
            identb = constp.tile([128, 128], BF16)
            nc.gpsimd.memset(identb, 0.0)
            nc.gpsimd.affine_select(out=identb, in_=identb,
                                    compare_op=mybir.AluOpType.not_equal,
                                    fill=1.0, base=0, pattern=[[-1, 128]],
                                    channel_multiplier=1)
            # causal additive mask for diagonal tiles: keep (p-f)>=0, else -1e9
            cmask = constp.tile([128, 128], F32)
            nc.gpsimd.memset(cmask, 0.0)
            nc.gpsimd.affine_select(out=cmask, in_=cmask,
                                    compare_op=mybir.AluOpType.is_ge,
                                    fill=NEG_BIG, base=0, pattern=[[-1, 128]],
                                    channel_multiplier=1)
            # cos/sin in token-major [tok%128, tok//128, D], scaled by 2^-WSHIFT
            cos_s = constp.tile([128, T // 128, D], F32)
            sin_s = constp.tile([128, T // 128, D], F32)
            nc.sync.dma_start(out=cos_s,
                              in_=cos_in[:].rearrange("(n p) d -> p n d", p=128))
            nc.sync.dma_start(out=sin_s,
                              in_=sin_in[:].rearrange("(n p) d -> p n d", p=128))
            comp = float(2.0 ** (-WSHIFT))
            nc.vector.tensor_scalar_mul(cos_s, cos_s, comp)
            nc.vector.tensor_scalar_mul(sin_s, sin_s, comp)

            # ---- resident quantized+transposed weights ----
            # wqkvT[kc]: [128 infeat, 768] (cols: 4 q-heads | k | v)
            wqkvT = [wres.tile([128, 6 * 128], FP8, tag=f"wqkvT{kc}", name=f"wqkvT{kc}")
                     for kc in range(NKC)]
            woT = [wres.tile([128, QH * D], FP8, tag=f"woT{kc}", name=f"woT{kc}")
                   for kc in range(NKC)]

            for rep in range(reps):
                with tc.tile_pool(name="psA", bufs=2, space="PSUM") as psA:
                    # ---------- hidden shard: quant -> transpose -> DRAM ----------
                    for pt in range(TPC // 128):           # 2 token part-tiles
                        for ch in range(4):                # free chunks of 1024
                            xt = stream.tile([128, 1024], F32, tag="hx")
                            nc.sync.dma_start(
                                out=xt, in_=hid_in[pt * 128:(pt + 1) * 128,
                                                  ch * 1024:(ch + 1) * 1024])
                            q8 = stream.tile([128, 1024], FP8, tag="hq8")
                            _emit_quant(nc, scratch, xt[:], q8[:], 3, CMAX_ACT, 0, "qq")
                            for t in range(8):
                                f0 = ch * 1024 + t * 128
                                tp = psA.tile([128, 128, 2], FP8, tag="tp8")
                                nc.tensor.transpose(tp[:, :, 0],
                                                    q8[:, t * 128:(t + 1) * 128], ident8)
                                ev = evac.tile([128, 128], FP8, tag="hev")
                                nc.vector.tensor_copy(out=ev, in_=tp[:, :, 0])
                                nc.sync.dma_start(
                                    out=hidT_loc[f0:f0 + 128, pt * 128:(pt + 1) * 128],
                                    in_=ev)

                    nc.gpsimd.collective_compute(
                        "AllGather", mybir.AluOpType.bypass,
                        replica_groups=[list(range(8))],
                        ins=[hidT_loc[:]], outs=[hidT_full[:]])

                    # ---------- weights: quant -> transpose -> resident SBUF ----------
                    # (wsrc, part-tile idx within tensor, dest-col offset in wqkvT)
                    wplan = ([(wq_in, m, m * 128) for m in range(QH)] +
                             [(wk_in, 0, 4 * 128), (wv_in, 0, 5 * 128)])
                    for (src, ptile, dcol) in wplan:
                        for ch in range(4):
                            xt = stream.tile([128, 1024], F32, tag="wx")
                            nc.sync.dma_start(
                                out=xt, in_=src[ptile * 128:(ptile + 1) * 128,
                                                ch * 1024:(ch + 1) * 1024])
                            q8 = stream.tile([128, 1024], FP8, tag="wq8")
                            _emit_quant(nc, scratch, xt[:], q8[:], 1, CMAX_W, WSHIFT, "qq")
                            for t in range(8):
                                kc = ch * 8 + t
                                tp = psA.tile([128, 128, 2], FP8, tag="tp8")
                                nc.tensor.transpose(tp[:, :, 0],
                                                    q8[:, t * 128:(t + 1) * 128], ident8)
                                nc.vector.tensor_copy(
                                    out=wqkvT[kc][:, dcol:dcol + 128], in_=tp[:, :, 0])
                    for mo in range(QH):
                        for ch in range(4):
                            xt = stream.tile([128, 1024], F32, tag="wx")
                            nc.sync.dma_start(
                                out=xt, in_=wo_in[mo * 128:(mo + 1) * 128,
                                                  ch * 1024:(ch + 1) * 1024])
                            q8 = stream.tile([128, 1024], FP8, tag="wq8")
                            _emit_quant(nc, scratch, xt[:], q8[:], 1, CMAX_W, WSHIFT, "qq")
                            for t in range(8):
                                kc = ch * 8 + t
                                tp = psA.tile([128, 128, 2], FP8, tag="tp8")
                                nc.tensor.transpose(tp[:, :, 0],
                                                    q8[:, t * 128:(t + 1) * 128], ident8)
                                nc.vector.tensor_copy(
                                    out=woT[kc][:, mo * 128:(mo + 1) * 128],
                                    in_=tp[:, :, 0])

                # ---------- QKV projections (token-major) + RoPE + transpose ----------
                qTr = [qkres.tile([128, T], BF16, tag=f"qTr{m}", name=f"qTr{m}") for m in range(QH)]
                kTr = qkres.tile([128, T], BF16, tag="kTr")
                v_all = qkres.tile([128, T // 128, 128], BF16, tag="v_all")

                with tc.tile_pool(name="psQ", bufs=2, space="PSUM") as psQ:
                    for tt in range(T // 128):             # 16 token tiles
                        pqkv = psQ.tile([128, 6 * 128], F32, tag="pqkv")
                        for kc in range(NKC):
                            lhs = stream.tile([128, 128], FP8, tag="hlhs")
                            r0 = (tt // 2) * H + kc * 128
                            c0 = (tt % 2) * 128
                            nc.sync.dma_start(out=lhs,
                                              in_=hidT_full[r0:r0 + 128, c0:c0 + 128])
                            st = (kc == 0)
                            sp = (kc == NKC - 1)
                            nc.tensor.matmul(pqkv[:, 0:512], lhs[:],
                                             wqkvT[kc][:, 0:512], start=st, stop=sp)
                            nc.tensor.matmul(pqkv[:, 512:768], lhs[:],
                                             wqkvT[kc][:, 512:768], start=st, stop=sp)
                        # v evac (token-major) with 2^-4 comp
                        nc.scalar.activation(out=v_all[:, tt, :], in_=pqkv[:, 640:768],
                                             func=mybir.ActivationFunctionType.Copy,
                                             scale=comp)
                        # RoPE (token-major: rotate_half is a free-dim shift)
                        rope_b = scratch.tile([128, 5, 128], BF16, tag="rope_b")
                        for r in range(5):                 # 4 q heads then k
                            c0 = r * 128
                            mfull = scratch.tile([128, 128], F32, tag="ropem")
                            nc.vector.tensor_tensor(out=mfull, in0=pqkv[:, c0:c0 + 128],
                                                    in1=cos_s[:, tt, :],
                                                    op=mybir.AluOpType.mult)
                            nlo = scratch.tile([128, 64], F32, tag="ropen")
                            nc.vector.tensor_tensor(out=nlo, in0=pqkv[:, c0 + 64:c0 + 128],
                                                    in1=sin_s[:, tt, 0:64],
                                                    op=mybir.AluOpType.mult)
                            nc.vector.tensor_tensor(out=rope_b[:, r, 0:64],
                                                    in0=mfull[:, 0:64], in1=nlo,
                                                    op=mybir.AluOpType.subtract)
                            nhi = scratch.tile([128, 64], F32, tag="ropen2")
                            nc.vector.tensor_tensor(out=nhi, in0=pqkv[:, c0:c0 + 64],
                                                    in1=sin_s[:, tt, 64:128],
                                                    op=mybir.AluOpType.mult)
                            nc.vector.tensor_tensor(out=rope_b[:, r, 64:128],
                                                    in0=mfull[:, 64:128], in1=nhi,
                                                    op=mybir.AluOpType.add)
                        # transpose rope outputs to [D, tok] bf16
                        for r in range(5):
                            tpb = psQ.tile([128, 128], BF16, tag="tpb")
                            nc.tensor.transpose(tpb, rope_b[:, r, :], identb)
                            dst = qTr[r] if r < QH else kTr
                            nc.vector.tensor_copy(out=dst[:, tt * 128:(tt + 1) * 128],
                                                  in_=tpb)

                # ---------- attention ----------
                with (
                    tc.tile_pool(name="psS", bufs=2, space="PSUM") as psS,
                    tc.tile_pool(name="psT", bufs=2, space="PSUM") as psT,
                ):
                    for b in range(B):
                        for h in range(QH):
                            for qt in range(8):
                                nk = (qt + 1) * 128
                                ps = psS.tile([128, 1024], F32, tag="scores")
                                lhs_q = qTr[h][:, b * S + qt * 128: b * S + (qt + 1) * 128]
                                for k2 in range((nk + 511) // 512):
                                    n2 = min(512, nk - k2 * 512)
                                    nc.tensor.matmul(
                                        ps[:, k2 * 512:k2 * 512 + n2], lhs_q,
                                        kTr[:, b * S + k2 * 512: b * S + k2 * 512 + n2],
                                        start=True, stop=True)
                                nc.vector.tensor_tensor(
                                    out=ps[:, qt * 128:nk], in0=ps[:, qt * 128:nk],
                                    in1=cmask, op=mybir.AluOpType.add)
                                rmax = scratch.tile([128, 1], F32, tag="rmax")
                                nc.vector.tensor_reduce(out=rmax, in_=ps[:, 0:nk],
                                                        axis=mybir.AxisListType.X,
                                                        op=mybir.AluOpType.max)
                                nbias = scratch.tile([128, 1], F32, tag="nbias")
                                nc.vector.tensor_scalar_mul(nbias, rmax, -SCALING)
                                probs = scratch.tile([128, 1024], BF16, tag="probs")
                                denom = scratch.tile([128, 1], F32, tag="denom")
                                nc.scalar.activation(out=probs[:, 0:nk], in_=ps[:, 0:nk],
                                                     func=mybir.ActivationFunctionType.Exp,
                                                     bias=nbias, scale=SCALING,
                                                     accum_out=denom)
                                recip = scratch.tile([128, 1], F32, tag="recip")
                                nc.vector.reciprocal(out=recip, in_=denom)
                                pa = psT.tile([128, 128], F32, tag="pattn")
                                for kt in range(qt + 1):
                                    ptp = psT.tile([128, 128], BF16, tag="tposm")
                                    nc.tensor.transpose(
                                        ptp, probs[:, kt * 128:(kt + 1) * 128], identb)
                                    pT = evac.tile([128, 128], BF16, tag="pTev")
                                    nc.vector.tensor_copy(out=pT, in_=ptp)
                                    nc.tensor.matmul(pa, pT,
                                                     v_all[:, b * 8 + kt, :],
                                                     start=(kt == 0), stop=(kt == qt))
                                att = scratch.tile([128, 128], F32, tag="attf")
                                nc.scalar.activation(out=att, in_=pa,
                                                     func=mybir.ActivationFunctionType.Copy,
                                                     scale=recip)
                                aq8 = scratch.tile([128, 128], FP8, tag="aq8")
                                _emit_quant(nc, scratch, att[:], aq8[:], 3, CMAX_ACT, 0, "qt")
                                tp = psT.tile([128, 128, 2], FP8, tag="tposm")
                                nc.tensor.transpose(tp[:, :, 0], aq8[:], ident8)
                                ev = evac.tile([128, 128], FP8, tag="aev")
                                nc.vector.tensor_copy(out=ev, in_=tp[:, :, 0])
                                nc.sync.dma_start(
                                    out=attnT_loc[b][h * 128:(h + 1) * 128,
                                                     qt * 128:(qt + 1) * 128],
                                    in_=ev)
                        nc.gpsimd.collective_compute(
                            "AllGather", mybir.AluOpType.bypass,
                            replica_groups=[list(range(8))],
                            ins=[attnT_loc[b][:]], outs=[attnT_full[b][:]])

                # ---------- output projection ----------
                with tc.tile_pool(name="psO", bufs=1, space="PSUM") as psO:
                    for b in range(B):
                        for tc2 in range(2):               # 512-token chunks in batch
                            po = [psO.tile([128, 512], F32, tag=f"po{mo}", name=f"po{mo}")
                                  for mo in range(QH)]
                            for kc in range(8 * QH):       # 32 chunks of 4096 attn feats
                                rhs = stream.tile([128, 512], FP8, tag="arhs")
                                nc.sync.dma_start(
                                    out=rhs,
                                    in_=attnT_full[b][kc * 128:(kc + 1) * 128,
                                                      tc2 * 512:(tc2 + 1) * 512])
                                for mo in range(QH):
                                    nc.tensor.matmul(po[mo][:],
                                                     woT[kc][:, mo * 128:(mo + 1) * 128],
                                                     rhs[:], start=(kc == 0),
                                                     stop=(kc == 8 * QH - 1))
                            for mo in range(QH):
                                osb = evac.tile([128, 512], F32, tag="oev")
                                nc.scalar.activation(out=osb, in_=po[mo][:],
                                                     func=mybir.ActivationFunctionType.Copy,
                                                     scale=comp)
                                nc.sync.dma_start(
                                    out=oT_out[mo * 128:(mo + 1) * 128,
                                               b * S + tc2 * 512: b * S + (tc2 + 1) * 512],
                                    in_=osb)

    nc.finalize()
    return nc


_NC_CACHE = {}


def _get_nc(reps=1):
    if reps not in _NC_CACHE:
        _NC_CACHE[reps] = build_kernel(reps)
    return _NC_CACHE[reps]


def make_in_maps(hidden_states, cos, sin, Wq, Wk, Wv, Wo):
    hs2 = np.ascontiguousarray(np.asarray(hidden_states, np.float32).reshape(T, H))
    cosR = np.ascontiguousarray(np.asarray(cos, np.float32).reshape(T, D))
    sinR = np.ascontiguousarray(np.asarray(sin, np.float32).reshape(T, D))
    Wq = np.asarray(Wq, np.float32)
    Wk = np.asarray(Wk, np.float32)
    Wv = np.asarray(Wv, np.float32)
    Wo = np.asarray(Wo, np.float32)
    in_maps = []
    for c in range(8):
        in_maps.append({
            "hid": np.ascontiguousarray(hs2[c * TPC:(c + 1) * TPC]),
            "wq": np.ascontiguousarray(Wq[c * 512:(c + 1) * 512]),
            "wk": np.ascontiguousarray(Wk[c * 128:(c + 1) * 128]),
            "wv": np.ascontiguousarray(Wv[c * 128:(c + 1) * 128]),
            "wo": np.ascontiguousarray(Wo[c * 512:(c + 1) * 512]),
            "cosR": cosR,
            "sinR": sinR,
        })
    return in_maps


def assemble(results):
    out = np.empty((T, H), np.float32)
    for c in range(8):
        out[:, c * 512:(c + 1) * 512] = results[c]["oT"].T
    return out.reshape(B, S, H)


def run_sharded(inputs, **kwargs):
    nc = _get_nc()
    in_maps = make_in_maps(
        inputs["hidden_states"], inputs["cos"], inputs["sin"],
        inputs["Wq"], inputs["Wk"], inputs["Wv"], inputs["Wo"])
    return run_bass_kernel_spmd(nc, in_maps, core_ids=list(range(8)), **kwargs)


def kernel(hidden_states, cos, sin, attention_mask, Wq, Wk, Wv, Wo):
    res = run_sharded({"hidden_states": hidden_states, "cos": cos, "sin": sin,
                       "Wq": Wq, "Wk": Wk, "Wv": Wv, "Wo": Wo})
    return assemble(res.results)

